# revision 17
# baseline (speedup 1.0000x reference)
"""Trainium2 Bass kernel for AdaptiveDiffusionConv (gnn_message_passing).

Reference computation (per batch b):
    a   = adj * att[b]                      # [m, n]
    S   = [I, a, a @ a]
    rhs[k] = S[k]^T @ x[b]                  # [n, (f,t)]
    out = relu(sum_k rhs[k] @ Theta[k])     # [n, (o,t)]

Reformulated (a@a never materialized; Theta commutes with the node-dim matmul):
    out = relu( x@Th0 + a^T (x@Th1 + a^T (x@Th2)) )
where x@Thk is the f-contraction, folded into the same PSUM accumulation
groups as the a^T matmuls by augmenting the contraction dim with (f,t)
rows: lhsT' = x^T[(f,t), n], rhs' = thblk[k] where
thblk[k][(f,t'), (o,t)] = Theta[k,f,o] * (t'==t)   (host-precomputed, bf16).

Sharding: pure data-parallel over batch B=16 across 8 cores (B_local=2).
adj / thblk / identity replicated; no collectives.
"""

import sys

sys.path.insert(0, "/opt/trn_rl_repo")

import numpy as np

import concourse.bacc as bacc
import concourse.mybir as mybir
from concourse import tile
from concourse.bass_utils import run_bass_kernel_spmd

B, N, F, T, K, O = 16, 1024, 16, 12, 3, 16
NCORES = 8
BL = B // NCORES  # 2 batches per core
P = 128
NT = N // P  # 8 node tiles
FT = F * T  # 192
OT = O * T  # 192
HC = FT // 2  # 96, contraction chunk for augmented rows

F32 = mybir.dt.float32
BF16 = mybir.dt.bfloat16
NP_BF16 = mybir.dt.np(BF16)

_CACHE = {}


def build_nc():
    nc = bacc.Bacc()

    x_ext = nc.declare_dram_parameter("x", [BL, N, F, T], F32, isOutput=False)
    att_ext = nc.declare_dram_parameter("att", [BL, N, N], F32, isOutput=False)
    adj_ext = nc.declare_dram_parameter("adj", [N, N], F32, isOutput=False)
    th_ext = nc.declare_dram_parameter("thblk", [HC, K * 2 * OT], BF16, isOutput=False)
    id_ext = nc.declare_dram_parameter("ident", [P, P], BF16, isOutput=False)
    out_ext = nc.declare_dram_parameter("out", [BL, N, O, T], F32, isOutput=True)

    x_tiled = x_ext.rearrange("b (i p) f t -> b p i (f t)", p=P)
    out_flat = out_ext.rearrange("b n o t -> b n (o t)")

    with tile.TileContext(nc) as tc:
        with (
            tc.tile_pool(name="const", bufs=1) as const,
            tc.tile_pool(name="big", bufs=1) as big,
            tc.tile_pool(name="psp", bufs=8, space="PSUM") as psp,
            tc.tile_pool(name="resp", bufs=8) as resp,
        ):
            # constants straight to SBUF in final bf16 layout (host-prepared)
            th = const.tile([HC, K * 2 * OT], BF16)
            nc.gpsimd.dma_start(th[:], th_ext[:])
            ident = const.tile([P, P], BF16)
            nc.gpsimd.dma_start(ident[:], id_ext[:])

            # persistent SBUF tensors
            adj_sb = big.tile([P, NT * N], F32)  # [128, 8*1024] f32
            att_sb = big.tile([P, BL * NT * N], F32)  # [128, 16*1024] f32
            a_sb = big.tile([P, BL * NT * N], BF16)  # [128, 16*1024] bf16
            xall = big.tile([P, BL * NT * FT], F32)  # [128, 3072] f32
            xbf = big.tile([P, BL * NT * FT], BF16)  # [128, 3072] bf16
            xT = big.tile([HC, BL * NT * 2 * P], BF16)  # [96, 4096] bf16
            vw = big.tile([P, BL * 2 * NT * OT], BF16)  # [128, 6144] bf16

            # ---- DMA issue order: ALL inputs on the sync queue, in exact
            # arrival order: x, (att0[j], adj[j]) pairs, att1[0..7].
            nc.sync.dma_start(
                xall[:].rearrange("p (b i m) -> p b i m", b=BL, i=NT),
                x_ext.rearrange("b (i p) f t -> p b i (f t)", p=P),
            )
            for j in range(NT):
                nc.sync.dma_start(
                    att_sb[:, j * N : (j + 1) * N], att_ext[0, j * P : (j + 1) * P, :]
                )
                nc.sync.dma_start(
                    adj_sb[:, j * N : (j + 1) * N], adj_ext[j * P : (j + 1) * P, :]
                )
            for j in range(NT):
                nc.sync.dma_start(
                    att_sb[:, (NT + j) * N : (NT + j + 1) * N],
                    att_ext[1, j * P : (j + 1) * P, :],
                )

            def a_slice(b, j, i):
                base = (b * NT + j) * N + i * P
                return a_sb[:, base : base + P]

            def xT_slice(b, i, c):
                base = ((b * NT + i) * 2 + c) * P
                return xT[:, base : base + P]

            def vw_slice(b, s, j):
                base = ((b * 2 + s) * NT + j) * OT
                return vw[:, base : base + OT]

            def th_slice(k, c):
                return th[:, (k * 2 + c) * OT : (k * 2 + c) * OT + OT]

            def mul_a(b, j):
                nc.vector.tensor_mul(
                    a_sb[:, (b * NT + j) * N : (b * NT + j) * N + N],
                    adj_sb[:, j * N : (j + 1) * N],
                    att_sb[:, (b * NT + j) * N : (b * NT + j) * N + N],
                )

            def cast_x(b, i):
                nc.scalar.copy(
                    xbf[:, (b * NT + i) * FT : (b * NT + i + 1) * FT],
                    xall[:, (b * NT + i) * FT : (b * NT + i + 1) * FT],
                )

            def transpose_x(b, i):
                tp = psp.tile([HC, 2 * P], BF16, tag="ps")
                xs = xbf[:, (b * NT + i) * FT : (b * NT + i) * FT + FT]
                for c in range(2):
                    nc.tensor.transpose(
                        tp[:, c * P : (c + 1) * P], xs[:, c * HC : (c + 1) * HC], ident[:]
                    )
                nc.scalar.copy(
                    xT[:, (b * NT + i) * 2 * P : (b * NT + i + 1) * 2 * P], tp[:]
                )

            def v2_tile(b, i):
                ps = psp.tile([P, OT], F32, tag="ps")
                for c in range(2):
                    nc.tensor.matmul(
                        ps[:], xT_slice(b, i, c), th_slice(2, c),
                        start=(c == 0), stop=(c == 1),
                    )
                nc.scalar.copy(vw_slice(b, 0, i)[:], ps[:])

            def w_groups(b, idxs, fillers):
                """j-ordered: psum groups for the given n-tiles open concurrently,
                a^T rank updates applied in m-tile arrival order. `fillers` is a
                list of callables providing PE work between j-blocks (each may
                allocate its own psum tile, so len(idxs) must leave slots free)."""
                pss = {}
                for i in idxs:
                    ps = psp.tile([P, OT], F32, tag="ps")
                    for c in range(2):
                        nc.tensor.matmul(
                            ps[:], xT_slice(b, i, c), th_slice(1, c),
                            start=(c == 0), stop=False,
                        )
                    pss[i] = ps
                fi = 0
                for j in range(NT):
                    for i in idxs:
                        nc.tensor.matmul(
                            pss[i][:], a_slice(b, j, i), vw_slice(b, 0, j),
                            start=False, stop=(j == NT - 1),
                        )
                    for _ in range(2):
                        if fi < len(fillers):
                            fillers[fi]()
                            fi += 1
                for f in fillers[fi:]:
                    f()
                for i in idxs:
                    nc.scalar.copy(vw_slice(b, 1, i)[:], pss[i][:])

            def out_tile(b, i):
                ps = psp.tile([P, OT], F32, tag="ps")
                for c in range(2):
                    nc.tensor.matmul(
                        ps[:], xT_slice(b, i, c), th_slice(0, c),
                        start=(c == 0), stop=False,
                    )
                for j in range(NT):
                    nc.tensor.matmul(
                        ps[:], a_slice(b, j, i), vw_slice(b, 1, j),
                        start=False, stop=(j == NT - 1),
                    )
                res = resp.tile([P, OT], F32, tag="res")
                nc.scalar.activation(res[:], ps[:], mybir.ActivationFunctionType.Relu)
                nc.scalar.dma_start(out_flat[b, i * P : (i + 1) * P, :], res[:])

            # ---- compute trace order ----
            for i in range(NT):
                cast_x(0, i)
                transpose_x(0, i)
                v2_tile(0, i)
            for j in range(NT):
                mul_a(0, j)

            # batch-1 transposes and v2 serve as PE filler between the
            # arrival-paced j-blocks of w(0)
            fillers = []
            for i in range(NT):
                fillers.append(lambda i=i: (cast_x(1, i), transpose_x(1, i)))
                fillers.append(lambda i=i: v2_tile(1, i))
            w_groups(0, list(range(6)), fillers)
            w_groups(0, [6, 7], [])
            for j in range(NT):
                mul_a(1, j)
            for i in range(NT):
                out_tile(0, i)
            w_groups(1, list(range(NT)), [])
            for i in range(NT):
                out_tile(1, i)

    nc.compile()
    return nc


def make_host_inputs(adj, Theta):
    thblk = np.zeros((K, FT, OT), np.float32)
    for t in range(T):
        rows = np.arange(F) * T + t
        cols = np.arange(O) * T + t
        for k in range(K):
            thblk[k][np.ix_(rows, cols)] = Theta[k]
    # device layout: [HC, K*2*OT] with th[:, (k*2+c)*OT:...] = thblk[k][c*HC:(c+1)*HC]
    th_dev = np.zeros((HC, K * 2 * OT), np.float32)
    for k in range(K):
        for c in range(2):
            th_dev[:, (k * 2 + c) * OT : (k * 2 + c) * OT + OT] = thblk[k][
                c * HC : (c + 1) * HC
            ]
    ident = np.eye(P, dtype=np.float32)
    return {
        "adj": np.ascontiguousarray(adj, np.float32),
        "thblk": th_dev.astype(NP_BF16),
        "ident": ident.astype(NP_BF16),
    }


def kernel(x, spatial_attention, adj, Theta):
    x = np.asarray(x, np.float32)
    att = np.asarray(spatial_attention, np.float32)
    adj = np.asarray(adj, np.float32)
    Theta = np.asarray(Theta, np.float32)

    if "nc" not in _CACHE:
        _CACHE["nc"] = build_nc()
    nc = _CACHE["nc"]

    shared = make_host_inputs(adj, Theta)
    in_maps = []
    for c in range(NCORES):
        in_maps.append(
            {
                "x": np.ascontiguousarray(x[c * BL : (c + 1) * BL]),
                "att": np.ascontiguousarray(att[c * BL : (c + 1) * BL]),
                **shared,
            }
        )
    res = run_bass_kernel_spmd(nc, in_maps, core_ids=list(range(NCORES)))
    out = np.concatenate([res.results[c]["out"] for c in range(NCORES)], axis=0)
    return out.astype(np.float32)
